# revision 1
# baseline (speedup 1.0000x reference)
"""Causal self-attention (GQA + RMS-norm + RoPE) Trainium2 Bass kernel.

Sharding: 8 cores = 4 batches x 2 head-groups (tensor-parallel over heads).
Core c = 2*b + t handles batch b with Q heads [8t, 8t+8) and KV heads
[2t, 2t+2). Each core computes a partial output projection (its heads'
rows of W_out); the host sums the two partials per batch.

All matmuls run as float32r (full fp32 data, full-rate PE mode).

Pipeline per core:
  P1: qkv = x @ W_shard (transposed-x input), RMS+RoPE on q/k in natural
      layout, PE-transpose q/k to [d, tok], spill qT/kT/v to DRAM scratch.
  P2: per 512-token query window, per head: scoresT = kT_tile.T @ qT_win,
      +tri-mask on diagonal tiles, exp (ACT, scale=hd^-0.5), then
      yT += v_tile.T @ expT and sums += ones.T @ expT; normalize yT by
      broadcasted 1/sums.
  P3: out = sum_h yT_norm_h.T @ W_out_h rows -> partial [S, D].
"""
import sys, os
sys.path.insert(0, '/opt/trn_rl_repo')
import numpy as np

from concourse import bass, bacc, mybir, tile

f32 = mybir.dt.float32
f32r = mybir.dt.float32r

B, S, D = 4, 2048, 2048
H, HKV, HD = 16, 4, 128
HLOC = H // 2          # 8 q heads per core
KVLOC = HKV // 2       # 2 kv heads per core
SCALE = float(HD) ** -0.5
RMS_EPS = float(np.finfo(np.float32).eps)
ROPE_BASE = 10000.0

NTC = S // 128         # 16 token tiles
NDT = D // 128         # 16 contraction tiles
NWIN = S // 512        # 4 query windows


def _rope_tables():
    inv_freq = (1.0 / (ROPE_BASE ** (np.arange(0, HD, 2, dtype=np.float32) / HD))).astype(np.float32)
    freqs = np.arange(S, dtype=np.float32)[:, None] * inv_freq[None, :]
    cos = np.cos(freqs).astype(np.float32)
    sin = np.sin(freqs).astype(np.float32)
    cos2 = np.concatenate([cos, cos], axis=1)        # [S, 128]
    sin2 = np.concatenate([sin, -sin], axis=1)       # [S, 128]
    return cos2, sin2


def _tri_masks():
    # mask[vi][p, f] = -1e30 where kv > q for scoresT diag tiles:
    # kv = 128*j + p, q = 512*w + f, vi = j - 4*w -> masked iff p + 128*vi > f
    m = np.zeros((4, 128, 512), dtype=np.float32)
    p = np.arange(128)[:, None]
    f = np.arange(512)[None, :]
    for vi in range(4):
        m[vi][(p + 128 * vi) > f] = -1e30
    return m


def _emit_rms_rope(nc, scr, psum_ap, nheads, cos1, sin1, nat_tile, eps_ap):
    """psum_ap: [128, nheads*128] qkv psum slice; writes RMS+RoPE result into
    nat_tile (SBUF). cos1/sin1: [128, 1, 128] APs (cos duplicated, [sin,-sin]).

    rot(q) = q*cos2 + swap_halves(q)*sin2;  out = rot(q) * rsqrt(mean(q^2)+eps)
    rsqrt computed as exp(-0.5*ln(ss/128+eps)) on ACT (DVE reciprocal is slow).
    """
    w = nheads * 128
    sq = scr.tile([128, w], f32, tag="sq")
    nc.scalar.activation(sq[:], psum_ap, mybir.ActivationFunctionType.Square)
    ss = scr.tile([128, nheads, 1], f32, tag="ss")
    nc.vector.tensor_reduce(
        ss[:], sq[:].rearrange("p (h f) -> p h f", h=nheads),
        axis=mybir.AxisListType.X, op=mybir.AluOpType.add)
    lg = scr.tile([128, nheads, 1], f32, tag="lg")
    nc.scalar.activation(lg[:], ss[:], mybir.ActivationFunctionType.Ln,
                         scale=1.0 / HD, bias=eps_ap)
    rinv = scr.tile([128, nheads, 1, 1], f32, tag="rinv")
    nc.scalar.activation(rinv[:], lg[:], mybir.ActivationFunctionType.Exp,
                         scale=-0.5)

    shp = [128, nheads, 2, 64]
    p4 = psum_ap.rearrange("p (h x f) -> p h x f", h=nheads, x=2)
    p4s = p4[:, :, ::-1, :]
    cb = cos1.rearrange("p t (x f) -> p t x f", x=2).to_broadcast(shp)
    sb_ = sin1.rearrange("p t (x f) -> p t x f", x=2).to_broadcast(shp)
    rb = rinv[:].to_broadcast(shp)
    t1 = scr.tile(shp, f32, tag="t1")
    t2 = scr.tile(shp, f32, tag="t2")
    nc.vector.tensor_mul(t1[:], p4, cb)
    nc.vector.tensor_mul(t2[:], p4s, sb_)
    nc.vector.tensor_add(t1[:], t1[:], t2[:])
    nc.vector.tensor_mul(nat_tile[:].rearrange("p (h x f) -> p h x f", h=nheads, x=2),
                         t1[:], rb)


def build_program():
    cos_np, sin_np = _rope_tables()
    masks_np = _tri_masks()

    nc = bacc.Bacc(trn_type="TRN2")

    xt_d = nc.dram_tensor("xt", [D, S], f32, kind="ExternalInput")
    wq_d = nc.dram_tensor("wq", [D, HLOC * HD], f32, kind="ExternalInput")
    wkv_d = nc.dram_tensor("wkv", [D, 2 * KVLOC * HD], f32, kind="ExternalInput")
    wo_d = nc.dram_tensor("wo", [HLOC * HD, D], f32, kind="ExternalInput")
    out_d = nc.dram_tensor("out", [S, D], f32, kind="ExternalOutput")

    cos_d = nc.inline_tensor(cos_np, "cos_t")
    sin_d = nc.inline_tensor(sin_np, "sin_t")
    ident_d = nc.inline_tensor(np.eye(128, dtype=np.float32), "ident")
    masks_d = nc.inline_tensor(masks_np, "tri_masks")
    onescol_d = nc.inline_tensor(np.ones((128, 1), dtype=np.float32), "onescol")
    onesrow_d = nc.inline_tensor(np.ones((1, 128), dtype=np.float32), "onesrow")

    qt_scr = nc.dram_tensor("qt_scr", [HLOC, 128, S], f32)
    kt_scr = nc.dram_tensor("kt_scr", [KVLOC, 128, S], f32)
    v_scr = nc.dram_tensor("v_scr", [S, KVLOC * HD], f32)

    with tile.TileContext(nc) as tc:
        with tc.tile_pool(name="cst", bufs=1) as cst:
            cos_sb = cst.tile([128, NTC, 128], f32, tag="cos")
            sin_sb = cst.tile([128, NTC, 128], f32, tag="sin")
            ident = cst.tile([128, 128], f32, tag="ident")
            masks = cst.tile([128, 4, 512], f32, tag="masks")
            ones = cst.tile([128, 1], f32r, tag="ones")
            ones_r = cst.tile([1, 128], f32r, tag="ones_r")
            eps_sb = cst.tile([128, 1], f32, tag="eps")
            nc.sync.dma_start(out=cos_sb[:], in_=cos_d[:].rearrange("(t p) f -> p t f", p=128))
            nc.sync.dma_start(out=sin_sb[:], in_=sin_d[:].rearrange("(t p) f -> p t f", p=128))
            nc.sync.dma_start(out=ident[:], in_=ident_d[:])
            nc.sync.dma_start(out=masks[:], in_=masks_d[:].rearrange("v p f -> p v f"))
            nc.sync.dma_start(out=ones[:], in_=onescol_d[:].bitcast(f32r))
            nc.sync.dma_start(out=ones_r[:], in_=onesrow_d[:].bitcast(f32r))
            nc.gpsimd.memset(eps_sb[:], RMS_EPS)

            # ---------------- Phase 1: QKV projection ----------------
            with tc.tile_pool(name="w1", bufs=1) as w1, \
                 tc.tile_pool(name="xs", bufs=3) as xs, \
                 tc.tile_pool(name="nat", bufs=3) as nat, \
                 tc.tile_pool(name="stg", bufs=4) as stg, \
                 tc.tile_pool(name="p1a", bufs=6, space="PSUM") as p1a, \
                 tc.tile_pool(name="p1t", bufs=2, space="PSUM") as p1t:

                wq_sb = w1.tile([128, NDT, HLOC * HD], f32r, tag="wq")
                wkv_sb = w1.tile([128, NDT, 512], f32r, tag="wkv")
                wq_r = wq_d[:].bitcast(f32r).rearrange("(t p) c -> p t c", p=128)
                wkv_r = wkv_d[:].bitcast(f32r).rearrange("(t p) c -> p t c", p=128)
                for dt in range(NDT):
                    nc.sync.dma_start(out=wkv_sb[:, dt, :], in_=wkv_r[:, dt, :])
                for dt in range(NDT):
                    nc.scalar.dma_start(out=wq_sb[:, dt, :], in_=wq_r[:, dt, :])

                for tcid in range(NTC):
                    xt_sb = xs.tile([128, NDT, 128], f32r, tag="xt")
                    nc.sync.dma_start(
                        out=xt_sb[:],
                        in_=xt_d[:, tcid * 128:(tcid + 1) * 128]
                            .bitcast(f32r).rearrange("(t p) s -> p t s", p=128))

                    ps_q1 = p1a.tile([128, 512], f32, tag="acc")
                    ps_q2 = p1a.tile([128, 512], f32, tag="acc")
                    ps_kv = p1a.tile([128, 512], f32, tag="acc")
                    for dt in range(NDT):
                        st, sp = dt == 0, dt == NDT - 1
                        nc.tensor.matmul(ps_kv[:], xt_sb[:, dt, :], wkv_sb[:, dt, :], start=st, stop=sp)
                    for dt in range(NDT):
                        st, sp = dt == 0, dt == NDT - 1
                        lhs = xt_sb[:, dt, :]
                        nc.tensor.matmul(ps_q1[:], lhs, wq_sb[:, dt, 0:512], start=st, stop=sp)
                        nc.tensor.matmul(ps_q2[:], lhs, wq_sb[:, dt, 512:1024], start=st, stop=sp)

                    cos1 = cos_sb[:, tcid:tcid + 1, :]
                    sin1 = sin_sb[:, tcid:tcid + 1, :]

                    # q heads 0-3 / 4-7: RMS+RoPE, then PE-transpose to qT
                    for gi, ps in ((0, ps_q1), (1, ps_q2)):
                        qn = nat.tile([128, 512], f32, tag="qn")
                        _emit_rms_rope(nc, nat, ps[:], 4, cos1, sin1, qn, eps_sb[:])
                        for hh in range(4):
                            h = gi * 4 + hh
                            tp = p1t.tile([128, 128], f32, tag="tp")
                            nc.tensor.transpose(tp[:], qn[:, hh * 128:(hh + 1) * 128], ident[:])
                            sg = stg.tile([128, 128], f32, tag="sg")
                            nc.vector.tensor_copy(sg[:], tp[:])
                            nc.scalar.dma_start(
                                out=qt_scr[h][:, tcid * 128:(tcid + 1) * 128], in_=sg[:])

                    # k heads (cols 0:256 of kv psum)
                    kn = nat.tile([128, 256], f32, tag="kn")
                    _emit_rms_rope(nc, nat, ps_kv[:, 0:256], 2, cos1, sin1, kn, eps_sb[:])
                    for kh in range(KVLOC):
                        tp = p1t.tile([128, 128], f32, tag="tp")
                        nc.tensor.transpose(tp[:], kn[:, kh * 128:(kh + 1) * 128], ident[:])
                        sg = stg.tile([128, 128], f32, tag="sg")
                        nc.vector.tensor_copy(sg[:], tp[:])
                        nc.scalar.dma_start(
                            out=kt_scr[kh][:, tcid * 128:(tcid + 1) * 128], in_=sg[:])

                    # v: plain copy out (natural layout)
                    vn = nat.tile([128, 256], f32, tag="vn")
                    nc.vector.tensor_copy(vn[:], ps_kv[:, 256:512])
                    nc.scalar.dma_start(
                        out=v_scr[tcid * 128:(tcid + 1) * 128, :], in_=vn[:])

            # ------------- Phases 2+3 (share the ytn resident) -------------
            with tc.tile_pool(name="ytp", bufs=1) as ytp:
                ytn = ytp.tile([128, HLOC, S], f32r, tag="ytn")

                # ---- Phase 2: attention ----
                with tc.tile_pool(name="kv2", bufs=1) as kv2, \
                     tc.tile_pool(name="qw", bufs=2) as qw, \
                     tc.tile_pool(name="ex", bufs=6) as ex, \
                     tc.tile_pool(name="sm", bufs=4) as sm, \
                     tc.tile_pool(name="p2s", bufs=3, space="PSUM") as p2s, \
                     tc.tile_pool(name="p2y", bufs=3, space="PSUM") as p2y, \
                     tc.tile_pool(name="p2n", bufs=2, space="PSUM") as p2n:

                    kt_sb = kv2.tile([128, KVLOC, S], f32r, tag="kt")
                    v_sb = kv2.tile([128, NTC, KVLOC * HD], f32r, tag="v")
                    nc.sync.dma_start(out=kt_sb[:], in_=kt_scr[:].bitcast(f32r).rearrange("k p t -> p k t"))
                    nc.sync.dma_start(out=v_sb[:], in_=v_scr[:].bitcast(f32r).rearrange("(t p) c -> p t c", p=128))

                    for w in range(NWIN):
                        qt_win = qw.tile([128, HLOC, 512], f32r, tag="qtw")
                        nc.sync.dma_start(
                            out=qt_win[:],
                            in_=qt_scr[:, :, w * 512:(w + 1) * 512].bitcast(f32r).rearrange("h p t -> p h t"))
                        njt = 4 * w + 4
                        for hq in range(HLOC):
                            kvh = hq // 4
                            ps_y = p2y.tile([128, 512], f32, tag="y")
                            ps_s = p2n.tile([1, 512], f32, tag="s")
                            rhs_q = qt_win[:, hq, :]
                            for j in range(njt):
                                ps_sc = p2s.tile([128, 512], f32, tag="sc")
                                nc.tensor.matmul(
                                    ps_sc[:],
                                    kt_sb[:, kvh, j * 128:(j + 1) * 128],
                                    rhs_q)
                                if j >= 4 * w:
                                    nc.vector.tensor_add(ps_sc[:], ps_sc[:], masks[:, j - 4 * w, :])
                                et = ex.tile([128, 512], f32r, tag="et")
                                nc.scalar.activation(et[:], ps_sc[:],
                                                     mybir.ActivationFunctionType.Exp,
                                                     scale=SCALE)
                                st, sp = j == 0, j == njt - 1
                                nc.tensor.matmul(
                                    ps_y[:],
                                    v_sb[:, j, kvh * 128:(kvh + 1) * 128],
                                    et[:], start=st, stop=sp,
                                    skip_group_check=True)
                                nc.tensor.matmul(
                                    ps_s[:], ones[:], et[:],
                                    start=st, stop=sp, skip_group_check=True)
                            lgs = sm.tile([1, 512], f32, tag="lgs")
                            nc.scalar.activation(lgs[:], ps_s[:],
                                                 mybir.ActivationFunctionType.Ln)
                            rec = sm.tile([1, 512], f32r, tag="rec")
                            nc.scalar.activation(rec[:], lgs[:],
                                                 mybir.ActivationFunctionType.Exp,
                                                 scale=-1.0)
                            bcp = p2s.tile([128, 512], f32, tag="sc")
                            nc.tensor.matmul(bcp[:], ones_r[:], rec[:])
                            bc = sm.tile([128, 512], f32, tag="bc")
                            nc.vector.tensor_copy(bc[:], bcp[:])
                            nc.vector.tensor_mul(
                                ytn[:, hq, w * 512:(w + 1) * 512], ps_y[:], bc[:])

                # ---- Phase 3: output projection ----
                with tc.tile_pool(name="w3", bufs=1) as w3, \
                     tc.tile_pool(name="ob", bufs=4) as ob, \
                     tc.tile_pool(name="p3", bufs=4, space="PSUM") as p3:
                    wo_sb = w3.tile([128, HLOC, D], f32r, tag="wo")
                    wo_r = wo_d[:].bitcast(f32r).rearrange("(h p) c -> p h c", p=128)
                    for og in range(4):
                        nc.sync.dma_start(out=wo_sb[:, :, og * 512:(og + 1) * 512],
                                          in_=wo_r[:, :, og * 512:(og + 1) * 512])
                    for og in range(4):
                        for tcid in range(NTC):
                            ps_o = p3.tile([128, 512], f32, tag="o")
                            for h in range(HLOC):
                                nc.tensor.matmul(
                                    ps_o[:],
                                    ytn[:, h, tcid * 128:(tcid + 1) * 128],
                                    wo_sb[:, h, og * 512:(og + 1) * 512],
                                    start=(h == 0), stop=(h == HLOC - 1))
                            ot = ob.tile([128, 512], f32, tag="ot")
                            nc.vector.tensor_copy(ot[:], ps_o[:])
                            nc.scalar.dma_start(
                                out=out_d[tcid * 128:(tcid + 1) * 128, og * 512:(og + 1) * 512],
                                in_=ot[:])

    nc.compile()
    return nc


_PROGRAM = None


def _get_program():
    global _PROGRAM
    if _PROGRAM is None:
        _PROGRAM = build_program()
    return _PROGRAM


def make_in_maps(x, W_qkv, W_out):
    in_maps = []
    for c in range(8):
        b, t = c // 2, c % 2
        xt = np.ascontiguousarray(x[b].T)
        wq = np.ascontiguousarray(W_qkv[:, t * 1024:(t + 1) * 1024])
        wk = W_qkv[:, D + t * 256: D + (t + 1) * 256]
        wv = W_qkv[:, D + 512 + t * 256: D + 512 + (t + 1) * 256]
        wkv = np.ascontiguousarray(np.concatenate([wk, wv], axis=1))
        wo = np.ascontiguousarray(W_out[t * 1024:(t + 1) * 1024, :])
        in_maps.append({"xt": xt, "wq": wq, "wkv": wkv, "wo": wo})
    return in_maps


def kernel(x, W_qkv, W_out):
    from concourse.bass_utils import run_bass_kernel_spmd
    nc = _get_program()
    in_maps = make_in_maps(np.asarray(x, dtype=np.float32),
                           np.asarray(W_qkv, dtype=np.float32),
                           np.asarray(W_out, dtype=np.float32))
    res = run_bass_kernel_spmd(nc, in_maps, list(range(8)), trace=False)
    out = np.empty((B, S, D), dtype=np.float32)
    for b in range(B):
        out[b] = res.results[2 * b]["out"] + res.results[2 * b + 1]["out"]
    return out



# revision 12
# speedup vs baseline: 1.5071x; 1.5071x over previous
"""Causal self-attention (GQA + RMS-norm + RoPE) Trainium2 Bass kernel.

Sharding: 8 cores = 4 batches x 2 head-groups (tensor-parallel over heads).
Core c = 2*b + t handles batch b with Q heads [8t, 8t+8) and KV heads
[2t, 2t+2). Each core computes a partial output projection (its heads'
rows of W_out); the host sums the two partials per batch.

v2 design (vs the phase-separated baseline):
  - All PE matmuls in bf16 (1 cyc/row, fast weight load). Host converts
    x / W_qkv / W_out shards to bf16; accumulation stays f32 in PSUM.
  - Single fused per-window pipeline (window = 512 query tokens):
      proj(w) -> attn(w) -> denom(w) -> [proj(w+1) fills PE] ->
      norm(w) + out-proj(w) -> attn(w+1) ...
    so the PE never idles long enough for the HAM clock gate to
    re-throttle it to 1.2 GHz.
  - qT/kT/v stay resident in SBUF (no DRAM spill round trip).
  - No Ln/Exp activation-table thrash: RMS rsqrt is a DVE Newton
    iteration; softmax denominators accumulate into one [8,512] PSUM
    bank via selector-matmuls and take ONE Ln + ONE Exp per window.
  - Causal diagonal tiles compute only the unmasked column range
    (width 512-128*vi), cutting score/exp/yacc work ~15%.
"""
import sys
sys.path.insert(0, '/opt/trn_rl_repo')
import numpy as np
import ml_dtypes

from concourse import bass, bacc, mybir, tile

f32 = mybir.dt.float32
bf16 = mybir.dt.bfloat16
BF = ml_dtypes.bfloat16

B, S, D = 4, 2048, 2048
H, HKV, HD = 16, 4, 128
HLOC = H // 2          # 8 q heads per core
KVLOC = HKV // 2       # 2 kv heads per core
SCALE = float(HD) ** -0.5
ROPE_BASE = 10000.0

NTC = S // 128         # 16 token tiles
NDT = D // 128         # 16 contraction tiles
NWIN = S // 512        # 4 query windows
AF = mybir.ActivationFunctionType
ALU = mybir.AluOpType


def _rope_tables():
    inv_freq = (1.0 / (ROPE_BASE ** (np.arange(0, HD, 2, dtype=np.float32) / HD))).astype(np.float32)
    freqs = np.arange(S, dtype=np.float32)[:, None] * inv_freq[None, :]
    cos = np.cos(freqs).astype(np.float32)
    sin = np.sin(freqs).astype(np.float32)
    cos2 = np.concatenate([cos, cos], axis=1)        # [S, 128]
    sin2 = np.concatenate([sin, -sin], axis=1)       # [S, 128]
    return cos2, sin2


def _tri_masks():
    # mask[vi][p, f] = -1e30 where kv > q on the 128-wide boundary strip of
    # diagonal tile vi: kv = 128*j + p, q = 512*w + 128*vi + f (strip starts
    # at column s0 = 128*vi), masked iff p > f.
    m = np.zeros((4, 128, 128), dtype=np.float32)
    p = np.arange(128)[:, None]
    f = np.arange(128)[None, :]
    for vi in range(4):
        m[vi][p > f] = -1e30
    return m


def build_program():
    cos_np, sin_np = _rope_tables()
    masks_np = _tri_masks()
    selS_np = np.tile(np.eye(8, dtype=BF)[None, :, :], (128, 1, 1))  # [128,h,i]=(i==h)
    selB_np = np.broadcast_to(np.eye(8, dtype=BF)[:, :, None], (8, 8, 128)).copy()

    nc = bacc.Bacc(trn_type="TRN2")

    xt_d = nc.dram_tensor("xt", [D, S], bf16, kind="ExternalInput")
    wqkv_d = nc.dram_tensor("wqkv", [D, 1536], bf16, kind="ExternalInput")
    wo_d = nc.dram_tensor("wo", [HLOC * HD, D], bf16, kind="ExternalInput")
    out_d = nc.dram_tensor("out", [S, D], f32, kind="ExternalOutput")

    cos_d = nc.inline_tensor(cos_np, "cos_t")
    sin_d = nc.inline_tensor(sin_np, "sin_t")
    ident_d = nc.inline_tensor(np.eye(128, dtype=BF), "ident")
    ident_f_d = nc.inline_tensor(np.eye(128, dtype=np.float32), "ident_f")
    masks_d = nc.inline_tensor(masks_np, "tri_masks")
    selS_d = nc.inline_tensor(selS_np, "selS")
    selB_d = nc.inline_tensor(selB_np, "selB")

    with tile.TileContext(nc) as tc:
        with tc.tile_pool(name="cst", bufs=1) as cst, \
             tc.tile_pool(name="xs", bufs=3) as xs, \
             tc.tile_pool(name="rms", bufs=2) as rms, \
             tc.tile_pool(name="nat", bufs=2) as nat, \
             tc.tile_pool(name="qp", bufs=2) as qp, \
             tc.tile_pool(name="yp", bufs=2) as yp, \
             tc.tile_pool(name="ep", bufs=3) as ep, \
             tc.tile_pool(name="dn", bufs=2) as dn, \
             tc.tile_pool(name="ob", bufs=3) as ob, \
             tc.tile_pool(name="pA", bufs=3, space="PSUM") as pA, \
             tc.tile_pool(name="pm", bufs=2, space="PSUM") as pm, \
             tc.tile_pool(name="pmt", bufs=1, space="PSUM") as pmt, \
             tc.tile_pool(name="py", bufs=1, space="PSUM") as py, \
             tc.tile_pool(name="ps8", bufs=1, space="PSUM") as ps8:

            # ---------------- constants / residents ----------------
            cos_sb = cst.tile([128, NTC, 128], f32, tag="cos")
            sin_sb = cst.tile([128, NTC, 128], f32, tag="sin")
            ident = cst.tile([128, 128], bf16, tag="ident")
            ident_f = cst.tile([128, 128], f32, tag="ident_f")
            masks = cst.tile([128, 4, 128], f32, tag="masks")
            selS = cst.tile([128, 8, 8], bf16, tag="selS")
            selB = cst.tile([8, 8, 128], bf16, tag="selB")
            wqkv_sb = cst.tile([128, NDT, 1536], bf16, tag="wqkv")
            wo_sb = cst.tile([128, HLOC, D], bf16, tag="wo")
            kt_all = cst.tile([128, KVLOC, S], bf16, tag="kt")
            v_all = cst.tile([128, NTC, KVLOC * HD], bf16, tag="v")

            nc.sync.dma_start(out=cos_sb[:], in_=cos_d[:].rearrange("(t p) f -> p t f", p=128))
            nc.sync.dma_start(out=sin_sb[:], in_=sin_d[:].rearrange("(t p) f -> p t f", p=128))
            nc.sync.dma_start(out=ident[:], in_=ident_d[:])
            nc.sync.dma_start(out=ident_f[:], in_=ident_f_d[:])
            nc.sync.dma_start(out=masks[:], in_=masks_d[:].rearrange("v p f -> p v f"))
            nc.sync.dma_start(out=selS[:], in_=selS_d[:])
            nc.sync.dma_start(out=selB[:], in_=selB_d[:])
            wqkv_r = wqkv_d[:].rearrange("(t p) c -> p t c", p=128)
            for dt in range(NDT):
                nc.sync.dma_start(out=wqkv_sb[:, dt, :], in_=wqkv_r[:, dt, :])

            # ---------------- per-tcid emission helpers ----------------
            def emit_proj_mm(tcid):
                """QKV projection matmuls for one 128-token tile."""
                xt_sb = xs.tile([128, NDT, 128], bf16, tag="xt", name="xt_sb")
                nc.sync.dma_start(
                    out=xt_sb[:],
                    in_=xt_d[:, tcid * 128:(tcid + 1) * 128]
                        .rearrange("(t p) s -> p t s", p=128))
                ps_kv = pA.tile([128, 512], f32, tag="acc", name="ps_kv")
                ps_q1 = pA.tile([128, 512], f32, tag="acc", name="ps_q1")
                ps_q2 = pA.tile([128, 512], f32, tag="acc", name="ps_q2")
                for dt in range(NDT):
                    st, sp = dt == 0, dt == NDT - 1
                    nc.tensor.matmul(ps_kv[:], xt_sb[:, dt, :], wqkv_sb[:, dt, 1024:1536], start=st, stop=sp)
                for dt in range(NDT):
                    st, sp = dt == 0, dt == NDT - 1
                    lhs = xt_sb[:, dt, :]
                    nc.tensor.matmul(ps_q1[:], lhs, wqkv_sb[:, dt, 0:512], start=st, stop=sp)
                    nc.tensor.matmul(ps_q2[:], lhs, wqkv_sb[:, dt, 512:1024], start=st, stop=sp)
                return ps_kv, ps_q1, ps_q2

            def emit_rope(tcid, psums):
                """RMS-norm (Newton rsqrt on DVE) + RoPE; returns nat tiles."""
                ps_kv, ps_q1, ps_q2 = psums
                # sum of squares: Square with scale 1/sqrt(HD) => mean after sum
                s = 1.0 / float(np.sqrt(HD))
                sq = rms.tile([128, 512], f32, tag="sq", name="sq")
                sst = rms.tile([128, 10, 1, 1], f32, tag="sst", name="sst")
                yt = rms.tile([128, 10, 1, 1], f32, tag="yt", name="yt")
                tt = rms.tile([128, 10, 1, 1], f32, tag="tt", name="tt")
                nc.scalar.activation(sq[:, 0:256], ps_kv[:, 0:256], AF.Square, scale=s)
                nc.vector.tensor_reduce(
                    sst[:, 8:10, 0, :], sq[:, 0:256].rearrange("p (h f) -> p h f", h=2),
                    axis=mybir.AxisListType.X, op=ALU.add)
                nc.scalar.activation(sq[:], ps_q1[:], AF.Square, scale=s)
                nc.vector.tensor_reduce(
                    sst[:, 0:4, 0, :], sq[:].rearrange("p (h f) -> p h f", h=4),
                    axis=mybir.AxisListType.X, op=ALU.add)
                nc.scalar.activation(sq[:], ps_q2[:], AF.Square, scale=s)
                nc.vector.tensor_reduce(
                    sst[:, 4:8, 0, :], sq[:].rearrange("p (h f) -> p h f", h=4),
                    axis=mybir.AxisListType.X, op=ALU.add)
                # Newton rsqrt: y0 = clamp(1.5 - 0.5 v, >= 0.2); 4 iters
                v = sst[:, :, 0, 0]
                y = yt[:, :, 0, 0]
                t = tt[:, :, 0, 0]
                nc.vector.tensor_scalar(y, v, -0.5, 1.5, ALU.mult, ALU.add)
                nc.vector.tensor_scalar_max(y, y, 0.2)
                for _ in range(4):
                    nc.vector.tensor_mul(t, y, y)
                    nc.vector.tensor_mul(t, t, v)
                    nc.vector.tensor_scalar(t, t, -0.5, 1.5, ALU.mult, ALU.add)
                    nc.vector.tensor_mul(y, y, t)

                cosr = cos_sb[:, tcid:tcid + 1, :].rearrange("p t (x f) -> p t x f", x=2)
                sinr = sin_sb[:, tcid:tcid + 1, :].rearrange("p t (x f) -> p t x f", x=2)

                def rope_group(ps_ap, nheads, rb4, out_tile):
                    shp = [128, nheads, 2, 64]
                    p4 = ps_ap.rearrange("p (h x f) -> p h x f", h=nheads, x=2)
                    p4s = p4[:, :, ::-1, :]
                    cb = cosr.to_broadcast(shp)
                    sb_ = sinr.to_broadcast(shp)
                    rb = rb4.to_broadcast(shp)
                    t1 = nat.tile([128, nheads, 2, 64], f32, tag=f"t1_{nheads}", name="t1")
                    t2 = nat.tile([128, nheads, 2, 64], f32, tag=f"t2_{nheads}", name="t2")
                    nc.vector.tensor_mul(t1[:], p4, cb)
                    nc.vector.tensor_mul(t2[:], p4s, sb_)
                    nc.vector.tensor_add(t1[:], t1[:], t2[:])
                    nc.vector.tensor_mul(
                        out_tile[:].rearrange("p (h x f) -> p h x f", h=nheads, x=2),
                        t1[:], rb)

                qn1 = nat.tile([128, 512], bf16, tag="qn1", name="qn1")
                qn2 = nat.tile([128, 512], bf16, tag="qn2", name="qn2")
                kn = nat.tile([128, 256], f32, tag="kn", name="kn")
                rope_group(ps_q1[:], 4, yt[:, 0:4], qn1)
                rope_group(ps_q2[:], 4, yt[:, 4:8], qn2)
                rope_group(ps_kv[:, 0:256], 2, yt[:, 8:10], kn)
                # v: plain copy to resident (bf16)
                nc.vector.tensor_copy(v_all[:, tcid, :], ps_kv[:, 256:512])
                return qn1, qn2, kn

            def emit_tr(tcid, nats, qt_w):
                """PE-transpose q/k for one tile into qt_w / kt_all."""
                qn1, qn2, kn = nats
                off = (tcid % 4) * 128
                tp = pmt.tile([128, 2, 512], bf16, tag="pmt", name="tp")
                for gi, qn in ((0, qn1), (1, qn2)):
                    for hh in range(4):
                        nc.tensor.transpose(tp[:, gi, hh * 128:(hh + 1) * 128],
                                            qn[:, hh * 128:(hh + 1) * 128], ident[:])
                    nc.vector.tensor_copy(
                        qt_w[:, gi * 4:(gi + 1) * 4, off:off + 128],
                        tp[:, gi, :].rearrange("p (h s) -> p h s", h=4))
                tpk = pm.tile([128, 512], f32, tag="pm", name="tpk")
                for kh in range(KVLOC):
                    nc.tensor.transpose(tpk[:, kh * 128:(kh + 1) * 128],
                                        kn[:, kh * 128:(kh + 1) * 128], ident_f[:])
                nc.vector.tensor_copy(
                    kt_all[:, :, tcid * 128:(tcid + 1) * 128],
                    tpk[:, 0:256].rearrange("p (h s) -> p h s", h=2))

            def emit_proj_window(w):
                """Full projection pipeline for window w; returns qt_w tile."""
                qt_w = qp.tile([128, HLOC, 512], bf16, tag="qtw", name="qt_w")
                nats_prev = None
                for i in range(4):
                    psums = emit_proj_mm(4 * w + i)
                    nats = emit_rope(4 * w + i, psums)
                    if nats_prev is not None:
                        emit_tr(4 * w + i - 1, nats_prev, qt_w)
                    nats_prev = nats
                emit_tr(4 * w + 3, nats_prev, qt_w)
                return qt_w

            def emit_attn_window(w, qt_w):
                """Attention for window w; returns (ytn_w, ps_s8_t) unnormalized."""
                njt = 4 * w + 4
                ytn_w = yp.tile([128, HLOC, 512], bf16, tag="ytn", name="ytn_w")
                ps_s8_t = ps8.tile([8, 512], f32, tag="s8", name="ps_s8")
                for hq in range(HLOC):
                    kvh = hq // 4
                    ps_y = py.tile([128, 512], f32, tag="y", name="ps_y")
                    for j in range(njt):
                        vi = j - 4 * w
                        s0 = 128 * vi if vi >= 0 else 0
                        ps_sc = pm.tile([128, 512], f32, tag="pm", name="ps_sc")
                        nc.tensor.matmul(
                            ps_sc[:, s0:512],
                            kt_all[:, kvh, j * 128:(j + 1) * 128],
                            qt_w[:, hq, s0:512])
                        if vi >= 0:
                            nc.vector.tensor_add(ps_sc[:, s0:s0 + 128],
                                                 ps_sc[:, s0:s0 + 128],
                                                 masks[:, vi, :])
                        et = ep.tile([128, 512], bf16, tag="et", name="et")
                        nc.scalar.activation(et[:, s0:512], ps_sc[:, s0:512],
                                             AF.Exp, scale=SCALE)
                        st, sp = j == 0, j == njt - 1
                        nc.tensor.matmul(
                            ps_y[:, s0:512],
                            v_all[:, j, kvh * 128:(kvh + 1) * 128],
                            et[:, s0:512], start=st, stop=sp,
                            skip_group_check=True)
                        nc.tensor.matmul(
                            ps_s8_t[:, s0:512], selS[:, hq, :], et[:, s0:512],
                            start=(hq == 0 and st), stop=(hq == HLOC - 1 and sp),
                            skip_group_check=True)
                    nc.vector.tensor_copy(ytn_w[:, hq, :], ps_y[:])
                return ytn_w, ps_s8_t

            def emit_denom(ps_s8_t):
                """One Ln + one Exp for all 8 heads' softmax denominators."""
                lg8 = dn.tile([8, 512], f32, tag="lg8", name="lg8")
                rec = dn.tile([8, 512], bf16, tag="rec", name="rec")
                nc.scalar.activation(lg8[:], ps_s8_t[:], AF.Ln)
                nc.scalar.activation(rec[:], lg8[:], AF.Exp, scale=-1.0)
                return rec

            def emit_norm_outproj(w, ytn_w, rec):
                """Normalize ytn by 1/denom, then project to output rows."""
                for hq in range(HLOC):
                    bcp = pm.tile([128, 512], f32, tag="pm", name="bcp")
                    nc.tensor.matmul(bcp[:], selB[:, hq, :], rec[:])
                    nc.vector.tensor_mul(ytn_w[:, hq, :], ytn_w[:, hq, :], bcp[:])
                for tc_i in range(4):
                    row0 = w * 512 + tc_i * 128
                    for og in range(4):
                        ps_o = pm.tile([128, 512], f32, tag="pm", name="ps_o")
                        for h in range(HLOC):
                            nc.tensor.matmul(
                                ps_o[:],
                                ytn_w[:, h, tc_i * 128:(tc_i + 1) * 128],
                                wo_sb[:, h, og * 512:(og + 1) * 512],
                                start=(h == 0), stop=(h == HLOC - 1))
                        ot = ob.tile([128, 512], f32, tag="ot", name="ot")
                        nc.vector.tensor_copy(ot[:], ps_o[:])
                        nc.gpsimd.dma_start(
                            out=out_d[row0:row0 + 128, og * 512:(og + 1) * 512],
                            in_=ot[:])

            # ---------------- main schedule ----------------
            qt_cur = emit_proj_window(0)
            pending = None  # (w, ytn_w, rec)
            for w in range(NWIN):
                ytn_w, ps_s8_t = emit_attn_window(w, qt_cur)
                rec = emit_denom(ps_s8_t)
                if w == 0:
                    # wo load can happen behind window-0 attention
                    wo_r = wo_d[:].rearrange("(h p) c -> p h c", p=128)
                    for og in range(4):
                        nc.sync.dma_start(out=wo_sb[:, :, og * 512:(og + 1) * 512],
                                          in_=wo_r[:, :, og * 512:(og + 1) * 512])
                if w < NWIN - 1:
                    qt_cur = emit_proj_window(w + 1)
                emit_norm_outproj(w, ytn_w, rec)

    nc.compile()
    return nc


_PROGRAM = None


def _get_program():
    global _PROGRAM
    if _PROGRAM is None:
        _PROGRAM = build_program()
    return _PROGRAM


def make_in_maps(x, W_qkv, W_out):
    x = np.asarray(x, dtype=np.float32)
    W_qkv = np.asarray(W_qkv, dtype=np.float32)
    W_out = np.asarray(W_out, dtype=np.float32)
    in_maps = []
    for c in range(8):
        b, t = c // 2, c % 2
        xt = np.ascontiguousarray(x[b].T).astype(BF)
        wq = W_qkv[:, t * 1024:(t + 1) * 1024]
        wk = W_qkv[:, D + t * 256: D + (t + 1) * 256]
        wv = W_qkv[:, D + 512 + t * 256: D + 512 + (t + 1) * 256]
        wqkv = np.ascontiguousarray(
            np.concatenate([wq, wk, wv], axis=1)).astype(BF)
        wo = np.ascontiguousarray(W_out[t * 1024:(t + 1) * 1024, :]).astype(BF)
        in_maps.append({"xt": xt, "wqkv": wqkv, "wo": wo})
    return in_maps


def kernel(x, W_qkv, W_out):
    from concourse.bass_utils import run_bass_kernel_spmd
    nc = _get_program()
    in_maps = make_in_maps(x, W_qkv, W_out)
    res = run_bass_kernel_spmd(nc, in_maps, list(range(8)), trace=False)
    out = np.empty((B, S, D), dtype=np.float32)
    for b in range(B):
        out[b] = res.results[2 * b]["out"] + res.results[2 * b + 1]["out"]
    return out
